# revision 10
# baseline (speedup 1.0000x reference)
"""Distributed Trainium2 kernel for a single attention head.

Problem: x:[8,2048,1024] f32, w_q/w_k/w_v:[1024,64] f32
  q,k,v = x@w ; scores = (q k^T)/sqrt(1024) causal-masked; out = softmax(scores)@v

Sharding: data-parallel over batch B=8 across the 8 NeuronCores (one batch
element per core, weights replicated, no collectives).

Per-core dataflow (T=2048, C=1024, H=64), exploiting PE quadrant packing:
two K=64 matmuls in row-groups 0/1 run concurrently (~2x) as long as each
PSUM accumulation group keeps a consistent tile position.
  - host ships x^T c-tiled [128, 8, T] bf16, packed wqkv [128, 8, 192] bf16,
    a multiplicative causal mask tile and a bf16 identity.
  - qk projection: contraction split into partition halves; h0 stream (row
    group 0) accumulates Pqk0, h1 stream (group 1) accumulates Pqk1,
    interleaved so they run concurrently; VectorE merges the partials
    (copy + add) into q/k SBUF tiles, dup'd onto both partition halves via
    SBUF DMA so score pairs can row-pack. v projection stays K=128 serial.
  - scores per s-tile pair run concurrently in row groups, into a 3-bank
    PSUM ring (slots stride-2; pair activation reads both slots via one
    strided AP). exp on ScalarE with scale=1/32 folded; causal diag-block
    masking via multiplicative mask on VectorE AFTER exp (keeps PE free).
  - PV row-packed: each s-tile's partition halves run concurrently into
    accH0/accH1 (lhsT = [v | 1] so row 64 accumulates the softmax
    denominator); merged by VectorE in the epilogue.
  - epilogue: bf16 PE transpose back to [t,h] (transposes share the proj
    PSUM banks via pool rotation), reciprocal-scale on VectorE, one tiled
    output DMA per chunk ([128, 4, 64] f32, host re-permutes).
  - x input: chunk 0 split across the 3 DMA queues for latency; chunks 1-3
    as half-chunk strided DMAs; next-chunk projection emission interleaved
    between score pairs to keep the PE queue dense.
"""

import os
import sys

import numpy as np

for p in ("/opt/trn_rl_repo",):
    if p not in sys.path and os.path.isdir(p):
        sys.path.insert(0, p)

import ml_dtypes  # noqa: E402

B, T, C, H = 8, 2048, 1024, 64
N_CORES = 8
TCH = 512                  # t-chunk (columns per PSUM bank of f32)
N_CHUNK = T // TCH         # 4
N_CT = C // 128            # 8 contraction tiles
SCALE = float(C) ** -0.5   # 1/32

_CACHE = {}


def _build():
    """Build + compile the SPMD Bass graph (same graph on all 8 cores)."""
    import concourse.bass as bass
    import concourse.mybir as mybir
    import concourse.tile as tile
    from concourse import bacc

    f32 = mybir.dt.float32
    bf16 = mybir.dt.bfloat16
    EXP = mybir.ActivationFunctionType.Exp
    BYP = mybir.AluOpType.bypass
    ADD = mybir.AluOpType.add
    MUL = mybir.AluOpType.mult

    nc = bacc.Bacc(
        "TRN2", target_bir_lowering=False, debug=False, num_devices=N_CORES
    )

    xT_d = nc.dram_tensor("xTt", [128, N_CT, T], bf16, kind="ExternalInput")
    wqkv_d = nc.dram_tensor("wqkv", [128, N_CT, 192], bf16, kind="ExternalInput")
    mask_d = nc.dram_tensor("maskb", [128, 128], bf16, kind="ExternalInput")
    idb_d = nc.dram_tensor("idb", [128, 128], bf16, kind="ExternalInput")
    out_d = nc.dram_tensor("outt", [128, T // 128, H], f32, kind="ExternalOutput")

    h0 = slice(0, 64)
    h1 = slice(64, 128)

    with tile.TileContext(nc) as tc:
        with (
            tc.tile_pool(name="const", bufs=1) as constp,
            tc.tile_pool(name="xTp", bufs=1) as xTp,
            tc.tile_pool(name="qkp", bufs=1) as qkp,
            tc.tile_pool(name="v1p", bufs=1) as v1p,
            tc.tile_pool(name="exp", bufs=6) as expp,
            tc.tile_pool(name="epi", bufs=2) as epip,
            tc.tile_pool(name="outb", bufs=2) as outp,
            tc.tile_pool(name="Sp", bufs=1, space="PSUM") as Sp,
            tc.tile_pool(name="projp", bufs=3, space="PSUM") as projp,
            tc.tile_pool(name="accp", bufs=1, space="PSUM") as accp,
        ):
            # ---- weights: c-tile 0 first (unblocks first matmul)
            wqkv_t = constp.tile([128, N_CT, 192], bf16, tag="wqkv", name="wqkv_t")
            nc.sync.dma_start(out=wqkv_t[:, 0, :], in_=wqkv_d[:, 0, :])

            mask_t = constp.tile([128, 128], bf16, tag="mask", name="mask_t")
            nc.gpsimd.dma_start(out=mask_t[:], in_=mask_d[:])
            idb_t = constp.tile([128, 128], bf16, tag="idb", name="idb_t")
            nc.gpsimd.dma_start(out=idb_t[:], in_=idb_d[:])

            # ---- x^T input feed (3 DMA queues: sync / scalar / gpsimd)
            xt = {}
            for t in range(N_CHUNK):
                xt[t] = xTp.tile([128, N_CT, TCH], bf16, tag=f"x{t}", name=f"x{t}")
            nc.sync.dma_start(out=xt[0][:, 0:2, :], in_=xT_d[:, 0:2, 0:TCH])
            nc.scalar.dma_start(out=xt[0][:, 2:5, :], in_=xT_d[:, 2:5, 0:TCH])
            nc.gpsimd.dma_start(out=xt[0][:, 5:8, :], in_=xT_d[:, 5:8, 0:TCH])
            nc.sync.dma_start(out=wqkv_t[:, 1:, :], in_=wqkv_d[:, 1:, :])
            nc.sync.dma_start(out=xt[1][:, 0:4, :], in_=xT_d[:, 0:4, TCH : 2 * TCH])
            nc.scalar.dma_start(
                out=xt[1][:, 4:8, :], in_=xT_d[:, 4:8, TCH : 2 * TCH]
            )
            nc.scalar.dma_start(
                out=xt[2][:, 0:4, :], in_=xT_d[:, 0:4, 2 * TCH : 3 * TCH]
            )
            nc.sync.dma_start(
                out=xt[2][:, 4:8, :], in_=xT_d[:, 4:8, 2 * TCH : 3 * TCH]
            )
            nc.sync.dma_start(out=xt[3][:, 0:4, :], in_=xT_d[:, 0:4, 3 * TCH :])
            nc.scalar.dma_start(out=xt[3][:, 4:8, :], in_=xT_d[:, 4:8, 3 * TCH :])

            # 3-bank score ring, single tile: slot-level deps come from
            # AP overlap tracking. Pair p uses slots (2p%3, (2p+1)%3).
            S_big = Sp.tile([128, 3, TCH], f32, tag="S", name="S_big")

            q2 = {}   # [128, TCH] bf16: qT duplicated on both partition halves
            k2 = {}   # [128, TCH] bf16: kT duplicated on both partition halves
            v1 = {}   # [128, 65] bf16 per s-tile: [v | 1]

            def proj_steps(tch):
                """Emission thunks for chunk `tch`'s projections."""
                steps = []
                st = {}

                def qk_slot(c):
                    def f():
                        if c == 0:
                            st["P0"] = projp.tile(
                                [128, TCH], f32, tag="pj", name=f"Pqk0_{tch}"
                            )
                            st["P1"] = projp.tile(
                                [128, TCH], f32, tag="pj", name=f"Pqk1_{tch}"
                            )
                        for hh, P in ((h0, st["P0"]), (h1, st["P1"])):
                            nc.tensor.matmul(
                                P[:],
                                wqkv_t[hh, c, 0:128],
                                xt[tch][hh, c, :],
                                start=(c == 0),
                                stop=(c == N_CT - 1),
                                skip_group_check=True,
                            )
                    return f

                def v_mm(c):
                    def f():
                        if c == 0:
                            st["Pv"] = projp.tile(
                                [64, TCH], f32, tag="pj", name=f"Pv_{tch}"
                            )
                        nc.tensor.matmul(
                            st["Pv"][:],
                            wqkv_t[:, c, 128:192],
                            xt[tch][:, c, :],
                            start=(c == 0),
                            stop=(c == N_CT - 1),
                            skip_group_check=True,
                        )
                    return f

                def qk_out():
                    qt = qkp.tile([128, TCH], bf16, tag=f"q2_{tch}", name=f"q2_{tch}")
                    kt = qkp.tile([128, TCH], bf16, tag=f"k2_{tch}", name=f"k2_{tch}")
                    nc.vector.tensor_copy(qt[0:64, :], st["P0"][0:64, :])
                    nc.vector.scalar_tensor_tensor(
                        qt[0:64, :], qt[0:64, :], 0.0, st["P1"][0:64, :], BYP, ADD
                    )
                    nc.vector.tensor_copy(kt[64:128, :], st["P0"][64:128, :])
                    nc.vector.scalar_tensor_tensor(
                        kt[64:128, :], kt[64:128, :], 0.0, st["P1"][64:128, :],
                        BYP, ADD,
                    )
                    nc.gpsimd.dma_start(out=qt[64:128, :], in_=qt[0:64, :])
                    nc.gpsimd.dma_start(out=kt[0:64, :], in_=kt[64:128, :])
                    q2[tch] = qt
                    k2[tch] = kt

                def v_out():
                    vTt = qkp.tile([64, TCH], bf16, tag=f"vT{tch}", name=f"vT{tch}")
                    nc.vector.tensor_copy(vTt[:], st["Pv"][:])
                    st["vT"] = vTt

                def v1_build(i):
                    def f():
                        j = 4 * tch + i
                        Pt = projp.tile([128, 64], bf16, tag="pj", name=f"Pt{j}")
                        nc.tensor.transpose(
                            Pt[:],
                            st["vT"][:, 128 * i : 128 * (i + 1)],
                            idb_t[0:64, 0:64],
                        )
                        v1t = v1p.tile([128, 65], bf16, tag=f"v1_{j}", name=f"v1_{j}")
                        nc.vector.tensor_copy(v1t[:, 0:64], Pt[:])
                        nc.vector.memset(v1t[:, 64:65], 1.0)
                        v1[j] = v1t
                    return f

                for c in range(N_CT):
                    steps.append(qk_slot(c))
                steps.append(qk_out)
                for c in range(N_CT):
                    steps.append(v_mm(c))
                steps.append(v_out)
                for i in range(4):
                    steps.append(v1_build(i))
                return steps

            # chunk-0 projection up front
            for s in proj_steps(0):
                s()

            slot = 0  # global S ring cursor
            for tch in range(N_CHUNK):
                pending = proj_steps(tch + 1) if tch + 1 < N_CHUNK else []
                jmax = 4 * tch + 3
                pairs = list(range(0, jmax + 1, 2))
                per_pair = -(-len(pending) // len(pairs)) if pending else 0

                aH = [
                    accp.tile([65, TCH], f32, tag="accH0", name=f"aH0_{tch}"),
                    accp.tile([65, TCH], f32, tag="accH1", name=f"aH1_{tch}"),
                ]
                for jp in pairs:
                    s0 = slot
                    s1 = (slot + 1) % 3
                    slot = (slot + 2) % 3
                    for jj, ss in ((0, s0), (1, s1)):
                        j = jp + jj
                        half = h1 if jj else h0
                        ksl = k2[j // 4][half, 128 * (j % 4) : 128 * (j % 4 + 1)]
                        nc.tensor.matmul(
                            S_big[:, ss, :],
                            ksl,
                            q2[tch][half, :],
                            start=True,
                            stop=True,
                            skip_group_check=True,
                        )
                    ext = expp.tile(
                        [128, 2, TCH], bf16, tag="ex", name=f"ex{tch}_{jp}"
                    )
                    if s1 == s0 + 1:
                        src = S_big[:, s0 : s0 + 2, :]
                    else:  # (2, 0) wraparound: negative slot stride
                        src = S_big[:, 2::-2, :]
                    nc.scalar.activation(ext[:], src, EXP, scale=SCALE)
                    # causal diag blocks: multiplicative upper-tri mask
                    # (s <= t survives), after exp, on VectorE
                    for jj in range(2):
                        j = jp + jj
                        rel = j - 4 * tch
                        if rel >= 0:
                            a = 128 * rel
                            nc.vector.scalar_tensor_tensor(
                                ext[:, jj, a : a + 128],
                                ext[:, jj, a : a + 128],
                                0.0,
                                mask_t[:],
                                BYP,
                                MUL,
                            )
                    # PV: partition halves row-packed into accH0/accH1
                    for jj in range(2):
                        j = jp + jj
                        lo = 128 * max(0, j - 4 * tch)
                        for hh in range(2):
                            nc.tensor.matmul(
                                aH[hh][:, lo:TCH] if j > 0 else aH[hh][:, :],
                                v1[j][64 * hh : 64 * (hh + 1), :],
                                ext[64 * hh : 64 * (hh + 1), jj, lo:TCH],
                                start=(j == 0),
                                stop=(j == jmax),
                                skip_group_check=True,
                            )
                    for _ in range(per_pair):
                        if pending:
                            pending.pop(0)()
                for s in pending:
                    s()

                # ======== epilogue: merge + normalize + transpose + DMA ====
                oT = epip.tile([65, TCH], bf16, tag="oT", name=f"oT{tch}")
                nc.vector.tensor_copy(oT[:], aH[0][:])
                nc.vector.scalar_tensor_tensor(
                    oT[:], oT[:], 0.0, aH[1][:], BYP, ADD
                )
                ob = outp.tile([128, 4, H], f32, tag="ob", name=f"ob{tch}")
                for i in range(4):
                    Pe = projp.tile([128, 65], bf16, tag="pj", name=f"Pe{tch}_{i}")
                    nc.tensor.transpose(
                        Pe[:],
                        oT[:, 128 * i : 128 * (i + 1)],
                        idb_t[0:65, 0:65],
                    )
                    rec = epip.tile([128, 1], f32, tag="rec", name=f"rec{tch}_{i}")
                    nc.vector.reciprocal(rec[:], Pe[:, 64:65])
                    nc.vector.tensor_scalar_mul(ob[:, i, :], Pe[:, 0:64], rec[:])
                nc.gpsimd.dma_start(
                    out=out_d[:, 4 * tch : 4 * tch + 4, :], in_=ob[:]
                )

    nc.compile()
    return nc


def _get_nc():
    if "nc" not in _CACHE:
        _CACHE["nc"] = _build()
    return _CACHE["nc"]


def _host_inputs(x, w_q, w_k, w_v):
    bf = ml_dtypes.bfloat16
    x = np.asarray(x, dtype=np.float32)
    wqkv = np.concatenate(
        [np.asarray(w_q, np.float32), np.asarray(w_k, np.float32),
         np.asarray(w_v, np.float32)], 1
    )
    wqkv_tiled = np.ascontiguousarray(
        wqkv.reshape(N_CT, 128, 192).transpose(1, 0, 2)
    ).astype(bf)
    # multiplicative causal mask for transposed-score diag blocks:
    # keep s <= t, i.e. partition p <= free f
    mask = np.ascontiguousarray(np.triu(np.ones((128, 128), np.float32))).astype(bf)
    idb = np.eye(128, dtype=np.float32).astype(bf)
    in_maps = []
    for i in range(N_CORES):
        xTt = np.ascontiguousarray(
            x[i].T.reshape(N_CT, 128, T).transpose(1, 0, 2)
        ).astype(bf)
        in_maps.append(
            {"xTt": xTt, "wqkv": wqkv_tiled, "maskb": mask, "idb": idb}
        )
    return in_maps


def run(x, w_q, w_k, w_v, trace=False, **trace_kwargs):
    from concourse.bass_utils import run_bass_kernel_spmd

    nc = _get_nc()
    in_maps = _host_inputs(x, w_q, w_k, w_v)
    res = run_bass_kernel_spmd(
        nc, in_maps, core_ids=list(range(N_CORES)), trace=trace, **trace_kwargs
    )
    outs = []
    for i in range(N_CORES):
        ot = np.asarray(res.results[i]["outt"])  # [128, 16, 64]
        outs.append(ot.transpose(1, 0, 2).reshape(T, H))
    return np.stack(outs).astype(np.float32), res


def kernel(x, w_q, w_k, w_v):
    out, _ = run(x, w_q, w_k, w_v, trace=False)
    return out
